# revision 34
# baseline (speedup 1.0000x reference)
"""Trainium2 Bass kernel for nn_PolyAttention (16-head polynomial causal attention).

Reference math (fp32):
    q = x @ Wq.T; k = x @ Wk.T; v = x @ Wv.T        (per-head dim 128, 16 heads)
    q, k = rope(q), rope(k)                          (LRPE type-1, base 10000)
    s = (q . k)^4, causal-masked, row-normalized by max(sum, 1e-6)
    out = (s @ v normalized) @ Wo.T
Sharding: 8 cores = batch(2) x head-group(4 heads each); host sums the 4
Wo partials per batch element.

v2 changes over the 383us-measured baseline (PE-bound at 93% occupancy),
measured 337.5us on the same setup:
  - denominator off the critical PE stream: the per-step M=1 ones-matmul
    (160 x 512-cycle streams) is replaced by batches of 4 col-group-tiled
    matmuls (tile_position=(0,32j), lhsT = [128,32] of 1/32) that run
    CONCURRENTLY on separate PE column strips -> ~1/4 the PE cycles
    (verified: batch starts 3ns apart on HW).  Group partials land in
    4x32 partition rows of one PSUM bank; per (head, qb): DVE copy to
    bf16 SBUF -> one [128,128]-ones fold matmul (all 128 output rows =
    exact total, the 1/32 lhsT cancels the 32x row redundancy; reuses the
    psd bank) -> deferred full-width DVE reciprocal.  Also removes the
    eps matmul (denominators >= 7.6e-3 >> 1e-6 so max(sum,eps)=sum) and
    the gpsimd partition_broadcast.  Batches stream s4 tiles LA steps
    behind the score pipeline (right after AV consumed them) so they
    never head-block the PE queue.
  - Wo chains split into per-(qt,jc) pieces (4 matmuls each) spread
    evenly through the next qb's attention via tick() -- the previous
    per-qt lumps let the ACT/DVE score pipeline drain and starved the PE.
  - scores emitted in kb-pairs sharing a 2-bank PSUM tile; fully
    off-diagonal pairs take one merged ACT square and one merged DVE
    quartic (halves per-op fixed + semaphore cost on those queues).
  - chunk-0 v2/v3 chains run inside attention(qb0)'s first steps (AV
    needs them only from step 6), filling the otherwise ACT-paced phase;
    k chains are waved like q (wk quarter granularity).
  - startup: first d-block pieces issued from the scalar/gpsimd queues in
    parallel with sync (each queue's first dma_start leaves only ~7us in,
    ~0.65us per issue), then growing piece sizes in first-use order;
    chunk-1 x is preloaded before wo.  Tail: last qb's Wo casts on the
    (then idle) scalar engine with per-jc output stores.
"""

import os
import sys

import numpy as np

if "/opt/trn_rl_repo" not in sys.path:
    sys.path.insert(0, "/opt/trn_rl_repo")

# ---------------------------------------------------------------- constants
B = 2
N = 2048
D = 2048
NH = 16
DH = 128
NHL = 4          # heads per core
HL = NHL * DH    # 512 local head dims
POLY = 4
LRPE_BASE = 10000.0

CH = 512         # projection n-chunk (columns of xT per step)
QB = 512         # query block
KB = 128         # key block
NDB = D // 128   # 16 contraction d-blocks
NCH = N // CH    # 4 chunks
NQB = N // QB    # 4 query blocks
LA = 4           # attention software-pipeline lookahead (score ahead of AV)


# ---------------------------------------------------------------- builder
def build_module(n=N):
    import concourse.bacc as bacc
    import concourse.mybir as mybir
    import concourse.tile as tile
    from concourse import bass_isa

    f32 = mybir.dt.float32
    bf16 = mybir.dt.bfloat16
    f16 = mybir.dt.float16
    AF = mybir.ActivationFunctionType

    nc = bacc.Bacc(
        "TRN2",
        target_bir_lowering=False,
        debug=False,
        enable_asserts=False,
        num_devices=8,
    )

    nch = n // CH
    nqb = n // QB

    # host-relayouted inputs (see make_in_maps)
    xt_d = nc.dram_tensor("xt", [128, nch, NDB * CH], bf16, kind="ExternalInput").ap()
    wq_d = nc.dram_tensor("wq", [128, NDB * HL], bf16, kind="ExternalInput").ap()
    wk_d = nc.dram_tensor("wk", [128, NDB * HL], bf16, kind="ExternalInput").ap()
    wv_d = nc.dram_tensor("wv", [128, NDB * HL], bf16, kind="ExternalInput").ap()
    wo_d = nc.dram_tensor("wo", [128, NHL * D], bf16, kind="ExternalInput").ap()
    cs_d = nc.dram_tensor("cs", [DH, n], f32, kind="ExternalInput").ap()
    sn_d = nc.dram_tensor("sn", [DH, n], f32, kind="ExternalInput").ap()
    mk_d = nc.dram_tensor("msk", [KB, KB], bf16, kind="ExternalInput").ap()
    out0_d = nc.dram_tensor("out0", [n, D // 2], f16, kind="ExternalOutput").ap()
    out1_d = nc.dram_tensor("out1", [n, D // 2], f16, kind="ExternalOutput").ap()

    def mm(out, lhsT, rhs, start, stop, tile_position=None, skip_group_check=False):
        nc.tensor.matmul(out, lhsT, rhs, start=start, stop=stop,
                         tile_position=tile_position,
                         skip_group_check=skip_group_check)

    with tile.TileContext(nc) as tc:
        from contextlib import ExitStack

        with ExitStack() as ctx:
            persist = ctx.enter_context(tc.tile_pool(name="persist", bufs=1))
            qTc = [persist.tile([128, NHL * CH], bf16, tag=f"qT{c}", name=f"qT{c}")
                   for c in range(nch)]
            kTc = [persist.tile([128, NHL * CH], bf16, tag=f"kT{c}", name=f"kT{c}")
                   for c in range(nch)]
            vSc = [persist.tile([128, (CH // 128) * HL], bf16, tag=f"vS{c}", name=f"vS{c}")
                   for c in range(nch)]
            nh2 = n // 2
            cs_t = [persist.tile([128, nh2], f32, tag=f"cs{i}", name=f"cs{i}") for i in range(2)]
            sn_t = [persist.tile([128, nh2], f32, tag=f"sn{i}", name=f"sn{i}") for i in range(2)]

            # PSUM: shps (2 banks) is shared by projection chains, Wo chains,
            # and 2 of every 5 score tiles; psS/psO/psD hold the other 6 banks.
            shps = ctx.enter_context(tc.tile_pool(name="shps", bufs=2, space="PSUM"))
            psP = ctx.enter_context(tc.tile_pool(name="c_pp", bufs=2, space="PSUM"))
            psO = ctx.enter_context(tc.tile_pool(name="c_po", bufs=1, space="PSUM"))
            psD = ctx.enter_context(tc.tile_pool(name="c_pd", bufs=1, space="PSUM"))

            wpool = ctx.enter_context(tc.tile_pool(name="w", bufs=1))
            xpool = ctx.enter_context(tc.tile_pool(name="x", bufs=2))
            tpool = ctx.enter_context(tc.tile_pool(name="t", bufs=2))
            s2pool = ctx.enter_context(tc.tile_pool(name="s2", bufs=2))
            s4pool = ctx.enter_context(tc.tile_pool(name="s4", bufs=4))
            rbpool = ctx.enter_context(tc.tile_pool(name="rb", bufs=2))
            dspool = ctx.enter_context(tc.tile_pool(name="ds", bufs=1))
            onpool = ctx.enter_context(tc.tile_pool(name="on", bufs=2))
            fopool = ctx.enter_context(tc.tile_pool(name="fo", bufs=2))

            wq_t = wpool.tile([128, NDB * HL], bf16, tag="wq", name="wq")
            wk_t = wpool.tile([128, NDB * HL], bf16, tag="wk", name="wk")
            wv_t = wpool.tile([128, NDB * HL], bf16, tag="wv", name="wv")
            wo_t = wpool.tile([128, NHL * D], bf16, tag="wo", name="wo")
            mk = wpool.tile([128, KB], bf16, tag="mk", name="mk")
            oneg = wpool.tile([128, 32], bf16, tag="oneg", name="oneg")
            nc.vector.memset(oneg[:], 1.0 / 32.0)
            ones128 = wpool.tile([128, 128], bf16, tag="ones128", name="ones128")
            nc.vector.memset(ones128[:], 1.0)
            xt_cs = [xpool.tile([128, NDB * CH], bf16, tag="xtc", name="xtc")
                     for c in range(nch)]

            # startup DMA order = exact first-use order; d-block granularity
            # for the first quarter (the DMA engines initialize staggered over
            # ~7us, so the first transfers complete slowly — keep them small),
            # eighths after.
            eig = NDB * HL // 8       # wq col-eighth (2 d-blocks, all heads)
            eig2 = NDB * CH // 8      # xt col-eighth
            qtr = NDB * HL // 4
            # first d-block pieces issued from the four compute-engine queues
            # in parallel: the sync queue's first dma_start only leaves at
            # ~7us (preamble) and each issue costs ~0.65us on its queue.
            nc.scalar.dma_start(wq_t[:, 0:HL], wq_d[:, 0:HL])
            nc.gpsimd.dma_start(xt_cs[0][:, 0:CH], xt_d[:, 0, 0:CH])
            nc.scalar.dma_start(wq_t[:, HL:2 * HL], wq_d[:, HL:2 * HL])
            nc.gpsimd.dma_start(xt_cs[0][:, CH:2 * CH], xt_d[:, 0, CH:2 * CH])
            nc.sync.dma_start(wq_t[:, 2 * HL:4 * HL], wq_d[:, 2 * HL:4 * HL])
            nc.sync.dma_start(xt_cs[0][:, 2 * CH:4 * CH], xt_d[:, 0, 2 * CH:4 * CH])
            for e in range(2, 4):
                nc.sync.dma_start(wq_t[:, e * eig:(e + 1) * eig], wq_d[:, e * eig:(e + 1) * eig])
                nc.sync.dma_start(xt_cs[0][:, e * eig2:(e + 1) * eig2], xt_d[:, 0, e * eig2:(e + 1) * eig2])
            nc.sync.dma_start(cs_t[0][:, 0:CH], cs_d[:, 0:CH])
            nc.sync.dma_start(sn_t[0][:, 0:CH], sn_d[:, 0:CH])
            for e in range(4, 8):
                nc.sync.dma_start(wq_t[:, e * eig:(e + 1) * eig], wq_d[:, e * eig:(e + 1) * eig])
                nc.sync.dma_start(xt_cs[0][:, e * eig2:(e + 1) * eig2], xt_d[:, 0, e * eig2:(e + 1) * eig2])
            for p in range(4):
                nc.sync.dma_start(wk_t[:, p * qtr:(p + 1) * qtr], wk_d[:, p * qtr:(p + 1) * qtr])
            nc.sync.dma_start(mk[:], mk_d[:, :])
            for p in range(4):
                nc.sync.dma_start(wv_t[:, p * qtr:(p + 1) * qtr], wv_d[:, p * qtr:(p + 1) * qtr])
            nc.sync.dma_start(cs_t[0][:, CH:], cs_d[:, CH:nh2])
            nc.sync.dma_start(sn_t[0][:, CH:], sn_d[:, CH:nh2])
            # chunk-1 x before wo: needed by the chunk-1 chains right after
            # attention(qb0); wo is first used only by qb0's Wo fillers which
            # run during qb1's attention.
            xhalf = NDB * CH // 2
            nc.sync.dma_start(xt_cs[1][:, 0:xhalf], xt_d[:, 1, 0:xhalf])
            nc.sync.dma_start(xt_cs[1][:, xhalf:], xt_d[:, 1, xhalf:])
            for p in range(2):
                nc.sync.dma_start(wo_t[:, p * NHL * D // 2:(p + 1) * NHL * D // 2],
                                  wo_d[:, p * NHL * D // 2:(p + 1) * NHL * D // 2])
            nc.sync.dma_start(cs_t[1][:], cs_d[:, nh2:])
            nc.sync.dma_start(sn_t[1][:], sn_d[:, nh2:])



            def chunk_dma(c):
                xt_c = xt_cs[c]
                if c > 1:
                    half = NDB * CH // 2
                    nc.sync.dma_start(xt_c[:, 0:half], xt_d[:, c, 0:half])
                    nc.sync.dma_start(xt_c[:, half:], xt_d[:, c, half:])

            def chunk_chains(c):
                """Return the projection-chain emitters for chunk c."""
                c0 = c * CH
                xt_c = xt_cs[c]
                csh = cs_t[(c0 // nh2)][:, c0 % nh2: c0 % nh2 + CH]
                snh = sn_t[(c0 // nh2)][:, c0 % nh2: c0 % nh2 + CH]
                out = []

                def rope_emit(ps, dstT, h):
                    dst = dstT[:, h * CH:(h + 1) * CH]
                    swp = tpool.tile([128, CH], f32, tag="swp", name="swp")
                    nc.scalar.copy(swp[0:64, :], ps[64:128, :])
                    nc.scalar.copy(swp[64:128, :], ps[0:64, :])
                    m1 = tpool.tile([128, CH], f32, tag="m1", name="m1")
                    nc.vector.tensor_mul(m1[:], ps[:], csh)
                    m2 = tpool.tile([128, CH], f32, tag="m2", name="m2")
                    nc.vector.tensor_mul(m2[:], swp[:], snh)
                    nc.vector.tensor_add(dst, m1[:], m2[:])

                def qk_chain(w_t, dstT, h):
                    ps = shps.tile([128, CH], f32, tag="ps", name="ps")
                    for i in range(NDB):
                        mm(ps[:], w_t[:, i * HL + h * 128: i * HL + (h + 1) * 128],
                           xt_c[:, i * CH:(i + 1) * CH],
                           start=(i == 0), stop=(i == NDB - 1))
                    rope_emit(ps[:], dstT, h)

                def v_chain(t2):
                    psv = shps.tile([128, HL], f32, tag="ps", name="psv")
                    for i in range(NDB):
                        mm(psv[:], xt_c[:, i * CH + t2 * 128: i * CH + (t2 + 1) * 128],
                           wv_t[:, i * HL:(i + 1) * HL],
                           start=(i == 0), stop=(i == NDB - 1))
                    nc.scalar.copy(vSc[c][:, t2 * HL:(t2 + 1) * HL], psv[:])

                if c == 0:
                    # chunk 0: q and k both run as quarter-K waves so the PE
                    # starts as soon as the first weight/x pieces land and
                    # never outruns the startup DMA stream.
                    qheld = []
                    kheld = []

                    qpp = []

                    def q_wave(w, d0, d1, held=qheld):
                        for h in range(NHL):
                            if w == 0 and d0 == 0:
                                if h % 2 == 0:
                                    qpp.append(psP.tile([128, 2 * QB], f32,
                                                        tag="pp", name="c0q"))
                                    ps = qpp[-1][:, 0:QB]
                                else:
                                    ps = qpp[-1][:, QB:]
                                held.append(ps)
                            ps = held[h]
                            for i in range(d0, d1):
                                mm(ps[:], wq_t[:, i * HL + h * 128: i * HL + (h + 1) * 128],
                                   xt_c[:, i * CH:(i + 1) * CH],
                                   start=(i == 0), stop=(i == NDB - 1))
                        if d1 == NDB:
                            for h in range(NHL):
                                rope_emit(held[h][:], qTc[0], h)

                    def k_wave(w, held=kheld):
                        i0, i1 = w * (NDB // 4), (w + 1) * (NDB // 4)
                        for h in range(NHL):
                            if w == 0:
                                if h < 2:
                                    ps = shps.tile([128, CH], f32, tag="ps", name="c0k")
                                elif h == 2:
                                    ps = psO.tile([128, QB], f32, tag="pso", name="c0k")
                                else:
                                    ps = psD.tile([128, QB], f32, tag="psd", name="c0k")
                                held.append(ps)
                            ps = held[h]
                            for i in range(i0, i1):
                                mm(ps[:, 0:CH], wk_t[:, i * HL + h * 128: i * HL + (h + 1) * 128],
                                   xt_c[:, i * CH:(i + 1) * CH],
                                   start=(i == 0), stop=(i == NDB - 1))
                        if w == 3:
                            for h in range(NHL):
                                rope_emit(held[h][:, 0:CH], kTc[0], h)

                    # first wave split into d-block pairs: first MM needs only
                    # 256KB of wq + 256KB of xt.
                    for dd in range(4):
                        out.append(lambda dd=dd: q_wave(0, dd, dd + 1))
                    for w in range(1, 4):
                        out.append(lambda w=w: q_wave(w, w * 4, (w + 1) * 4))
                    for w in range(4):
                        out.append(lambda w=w: k_wave(w))
                    # v2/v3 run inside attention(qb0)'s first steps (AV needs
                    # them only from step 6/7) to fill the ACT-paced phase.
                    for t2 in range(2):
                        out.append(lambda t2=t2: v_chain(t2))
                    pre = [lambda: v_chain(2), lambda: v_chain(3)]
                    return out, pre
                else:
                    for h in range(NHL):
                        out.append(lambda h=h: qk_chain(wq_t, qTc[c], h))
                    for h in range(NHL):
                        out.append(lambda h=h: qk_chain(wk_t, kTc[c], h))
                for t2 in range(CH // 128):
                    out.append(lambda t2=t2: v_chain(t2))
                return out, []

            pend = []       # deferred recip+normalize emission (cross-head)

            def emit_attention(qb, fillers, pre=()):
                nkb = (qb + 1) * (QB // KB)
                nbatch = nkb // 4
                steps_total = NHL * (nkb + LA)
                nf = len(fillers)
                fi = [0]
                sg = [0]

                def tick():
                    while fi[0] < nf and sg[0] >= (fi[0] + 1) * steps_total // (nf + 1):
                        fillers[fi[0]]()
                        fi[0] += 1
                    sg[0] += 1
                onrm = [onpool.tile([128, QB], bf16, tag=f"onrm{h}", name=f"onrm{h}")
                        for h in range(NHL)]
                for h in range(NHL):
                    pso = psO.tile([128, QB], f32, tag="pso", name="pso")
                    psd4 = psD.tile([128, QB], f32, tag="psd", name="psd")
                    s4q = {}
                    s4done = []
                    for step in range(nkb + LA):
                        if step == 1 and pend:
                            pend.pop()()
                        if h == 0 and step < len(pre):
                            pre[step]()
                        tick()
                        if step < nkb and step % 2 == 0:
                            # scores in kb-PAIRS sharing a 2-bank PSUM tile so
                            # fully-off-diagonal pairs take ONE merged square
                            # and ONE merged quartic (halves the per-op fixed
                            # + semaphore cost on the ACT/DVE queues).
                            kb0, kb1 = step, step + 1
                            rel1 = kb1 - qb * (QB // KB)
                            pp = psP.tile([128, 2 * QB], f32, tag="pp", name="pp")
                            s2 = s2pool.tile([128, 2 * QB], bf16, tag="s2", name="s2")
                            s4 = s4pool.tile([128, 2 * QB], bf16, tag="s4", name="s4")
                            halves = []
                            for ofs, kb in ((0, kb0), (QB, kb1)):
                                rel = kb - qb * (QB // KB)
                                cr = 0 if rel < 0 else 128 * rel
                                halves.append((ofs, kb, cr, rel))
                                kc, kr = kb // (CH // KB), kb % (CH // KB)
                                mm(pp[:, ofs + cr:ofs + QB],
                                   kTc[kc][:, h * CH + kr * KB: h * CH + (kr + 1) * KB],
                                   qTc[qb][:, h * CH + cr:(h + 1) * CH],
                                   start=True, stop=True)
                            if rel1 < 0:
                                nc.scalar.activation(s2[:], pp[:], AF.Square)
                                nc.vector.tensor_mul(s4[:], s2[:], s2[:])
                            else:
                                for ofs, kb, cr, rel in halves:
                                    nc.scalar.activation(s2[:, ofs + cr:ofs + QB],
                                                         pp[:, ofs + cr:ofs + QB],
                                                         AF.Square)
                                    if rel >= 0:
                                        nc.vector.tensor_mul(
                                            s2[:, ofs + cr:ofs + cr + 128],
                                            s2[:, ofs + cr:ofs + cr + 128], mk[:])
                                    if qb == 0 and cr > 0:
                                        # qb0's only denom batch is diagonal:
                                        # zero the masked prefix so its
                                        # full-width streams are valid.
                                        nc.vector.memset(s4[:, ofs:ofs + cr], 0.0)
                                    nc.vector.tensor_mul(s4[:, ofs + cr:ofs + QB],
                                                         s2[:, ofs + cr:ofs + QB],
                                                         s2[:, ofs + cr:ofs + QB])
                            for ofs, kb, cr, rel in halves:
                                s4q[kb] = (s4, ofs, cr)
                        if step >= LA:
                            kb = step - LA
                            s4, ofs, cr = s4q.pop(kb)
                            kc, kr = kb // (CH // KB), kb % (CH // KB)
                            mm(pso[:, cr:],
                               vSc[kc][:, kr * HL + h * 128: kr * HL + (h + 1) * 128],
                               s4[:, ofs + cr:ofs + QB],
                               start=(kb == 0), stop=(kb == nkb - 1))
                            s4done.append((kb, s4, ofs, cr))
                            if len(s4done) == 4:
                                # denominator batch: 4 col-group-tiled M=32
                                # matmuls (concurrent strips).  lhsT is 1/32
                                # everywhere so the 32x row redundancy cancels
                                # in the later 128-partition fold matmul.
                                bidx = s4done[0][0] // 4
                                for j, (kbj, s4j, ofsj, crj) in enumerate(s4done):
                                    c0j = 0 if qb == 0 else crj
                                    mm(psd4[32 * j:32 * (j + 1), c0j:],
                                       oneg[:, :], s4j[:, ofsj + c0j:ofsj + QB],
                                       start=(bidx == 0), stop=(bidx == nbatch - 1),
                                       tile_position=(0, 32 * j),
                                       skip_group_check=True)
                                s4done.clear()
                    # fold the 4x32 group-partial rows to the exact total in
                    # every partition row: one [128,128]-ones matmul (the 1/32
                    # lhsT scaling cancels the 32x row redundancy).  psfold
                    # reuses psd4's bank (same pool tag) and is released by
                    # the deferred reciprocal.
                    dsb = dspool.tile([128, QB], bf16, tag="dsb", name="dsb")
                    if qb == nqb - 1 and h == NHL - 1:
                        # tail-critical: the scalar engine is idle here and
                        # shaves ~0.2us off the final normalize chain.
                        nc.scalar.copy(dsb[:], psd4[:])
                    else:
                        nc.vector.tensor_copy(dsb[:], psd4[:])
                    psfold = psD.tile([128, QB], f32, tag="psd", name="psfold")
                    rbc = rbpool.tile([128, QB], f32, tag="rbc", name="rbc")

                    # the fold matmul is deferred with the normalize so it
                    # never sits at the PE queue head waiting for the dsb
                    # copy at a head boundary.
                    def _norm(h=h, pso=pso, dsb=dsb, psfold=psfold, rbc=rbc,
                              onrm=onrm):
                        mm(psfold[:], ones128[:, :], dsb[:], start=True, stop=True)
                        nc.vector.reciprocal_approx_fast(rbc[:], psfold[:])
                        nc.vector.tensor_mul(onrm[h][:], pso[:], rbc[:])
                    pend.append(_norm)

                # Wo chains for this qb run as fine-grained PE fillers during
                # the NEXT qb's attention: one piece per (qt, jc) = 4 matmuls
                # + cast, spread evenly by tick() so the ACT/DVE score
                # pipeline never drains behind a long Wo lump.
                fouts = {}

                def emit_wo_piece(qt, jc, qb=qb, onrm=onrm, fouts=fouts):
                    if jc == 0:
                        fouts[qt] = fopool.tile([128, D], f16, tag="fout", name="fout")
                    fout = fouts[qt]
                    r0 = qb * QB + qt * 128
                    psf = shps.tile([128, 512], f32, tag="ps", name="psf")
                    for h in range(NHL):
                        mm(psf[:], onrm[h][:, qt * 128:(qt + 1) * 128],
                           wo_t[:, h * D + jc * 512: h * D + (jc + 1) * 512],
                           start=(h == 0), stop=(h == NHL - 1))
                    fo = fout[:, jc * 512:(jc + 1) * 512]
                    if qb == nqb - 1:
                        # tail-critical: cast on the (idle) scalar engine and
                        # store each jc column block as soon as it's ready.
                        nc.scalar.copy(fo, psf[:])
                        if jc < 2:
                            nc.sync.dma_start(
                                out0_d[r0:r0 + 128, jc * 512:(jc + 1) * 512], fo)
                        else:
                            nc.sync.dma_start(
                                out1_d[r0:r0 + 128, (jc - 2) * 512:(jc - 1) * 512], fo)
                    else:
                        nc.vector.tensor_copy(fo, psf[:])
                        if jc == 1:
                            nc.sync.dma_start(out0_d[r0:r0 + 128, :], fout[:, 0:D // 2])
                        elif jc == 3:
                            nc.sync.dma_start(out1_d[r0:r0 + 128, :], fout[:, D // 2:])

                return [lambda qt=qt, jc=jc: emit_wo_piece(qt, jc)
                        for qt in range(QB // 128) for jc in range(D // 512)]

            wo_fillers = []
            for c in range(nch):
                chains, pre = chunk_chains(c)
                for f in chains:
                    f()
                if c + 1 < nch:
                    chunk_dma(c + 1)
                wo_fillers = emit_attention(c, wo_fillers, pre)
            while pend:
                pend.pop()()
            for f in wo_fillers:
                f()

    nc.compile()
    return nc


# ---------------------------------------------------------------- host prep
def _rope_tables(n):
    half = DH // 2
    theta = LRPE_BASE ** (-np.arange(half, dtype=np.float64) * 2.0 / DH)
    pos = np.arange(n, dtype=np.float64)
    ang = np.outer(pos, theta)                       # [n, 64]
    cos = np.cos(ang).T.astype(np.float32)           # [64, n]
    sin = np.sin(ang).T.astype(np.float32)
    cs = np.concatenate([cos, cos], axis=0)          # [128, n]
    sn = np.concatenate([-sin, sin], axis=0)
    return np.ascontiguousarray(cs), np.ascontiguousarray(sn)


def _mask():
    kp = np.arange(KB)[:, None]
    j = np.arange(KB)[None, :]
    return (kp <= j).astype(np.float32)


def make_in_maps(x, Wq, Wk, Wv, Wo, n=N):
    import ml_dtypes
    bf16 = ml_dtypes.bfloat16

    cs, sn = _rope_tables(n)
    mk = _mask()
    nch = n // CH

    def relayout_x(xb):
        # xt [d, n] -> [128, nch, NDB*CH]: chunk c contiguous per partition
        xt = np.ascontiguousarray(xb.T)
        a = xt.reshape(NDB, 128, nch, CH).transpose(1, 2, 0, 3)
        return np.ascontiguousarray(a.reshape(128, nch, NDB * CH).astype(bf16))

    def relayout_w(Wrows):
        # W[rows,:].T [d, HL] -> [128, NDB*HL]
        w = Wrows.T.reshape(NDB, 128, HL).transpose(1, 0, 2)
        return np.ascontiguousarray(w.reshape(128, NDB * HL).astype(bf16))

    def relayout_wo(Wcols):
        # Wo[:, rows].T [HL, D] -> [128, NHL*D]
        w = Wcols.T.reshape(NHL, 128, D).transpose(1, 0, 2)
        return np.ascontiguousarray(w.reshape(128, NHL * D).astype(bf16))

    xts = [relayout_x(x[b]) for b in range(x.shape[0])]
    in_maps = []
    for core in range(8):
        b, g = core // 4, core % 4
        rows = slice(g * HL, (g + 1) * HL)
        in_maps.append({
            "xt": xts[b],
            "wq": relayout_w(Wq[rows, :]),
            "wk": relayout_w(Wk[rows, :]),
            "wv": relayout_w(Wv[rows, :]),
            "wo": relayout_wo(Wo[:, rows]),
            "cs": cs,
            "sn": sn,
            "msk": mk.astype(bf16),
        })
    return in_maps


_NC_CACHE = {}


def _get_nc(n=N):
    if n not in _NC_CACHE:
        _NC_CACHE[n] = build_module(n)
    return _NC_CACHE[n]


def run(x, Wq, Wk, Wv, Wo, trace=False, **kw):
    from concourse.bass_utils import run_bass_kernel_spmd

    x = np.asarray(x, dtype=np.float32)
    nc = _get_nc(x.shape[1])
    in_maps = make_in_maps(x, Wq, Wk, Wv, Wo, n=x.shape[1])
    res = run_bass_kernel_spmd(nc, in_maps, core_ids=list(range(8)), trace=trace, **kw)
    outs = [np.concatenate([np.asarray(res.results[i]["out0"], dtype=np.float32),
                            np.asarray(res.results[i]["out1"], dtype=np.float32)], axis=1)
            for i in range(8)]
    b0 = outs[0] + outs[1] + outs[2] + outs[3]
    b1 = outs[4] + outs[5] + outs[6] + outs[7]
    out = np.stack([b0, b1]).astype(np.float32)
    return out, res


def kernel(x, Wq, Wk, Wv, Wo):
    out, _ = run(
        np.asarray(x, np.float32),
        np.asarray(Wq, np.float32),
        np.asarray(Wk, np.float32),
        np.asarray(Wv, np.float32),
        np.asarray(Wo, np.float32),
    )
    return out
